# revision 20
# baseline (speedup 1.0000x reference)
"""Distributed Trainium2 kernel for nn_Attn (sparse_attention softmax-GEMV).

Computes: softmax(encoder_states @ (W_attn @ (W_lin @ hidden + b_lin) + b_attn))[:, None]

Strategy (8 NeuronCores):
- encoder_states [32768, 1024] row-sharded: 4096 rows/core (16 MB/core, the
  memory-bound part).
- W_lin / W_attn row-sharded 128 rows/core; each core computes its 128
  elements of h = W_lin@hidden+b_lin exactly, AllGather -> h, same for
  energy = W_attn@h+b_attn, AllGather -> energy.
- Big GEMV e = enc @ energy runs on VectorE as fused scalar_tensor_tensor
  tiles [128, 1024] (mult + free-axis accumulate in one pass), overlapped
  with the streaming HBM DMA of enc.
- Softmax: flash-style. Each core computes local max m_c and local
  sum s_c = sum exp(e - m_c), AllGathers the (m_c, s_c) pairs, then derives
  the global max/denominator and rescales locally. One collective in the
  tail, no cross-core max/sum asymmetry.
- Cross-partition reductions/broadcasts use TensorE (ones-matmul, transpose
  with identity), never GPSIMD.
"""

import sys

if "/opt/trn_rl_repo" not in sys.path:
    sys.path.insert(0, "/opt/trn_rl_repo")

import numpy as np

H = 1024
S = 32768
NCORES = 8
S_LOC = S // NCORES          # 4096 rows of encoder_states per core
RT = S_LOC // 128            # 32 row-tiles per core
WROWS = H // NCORES          # 128 rows of each weight matrix per core
DTILES = S_LOC // 256        # double-row tiles (8KB/partition DMA descriptors)
CHUNK_DTILES = 4             # dtiles per enc DMA (4 * 1MB = 4MB)
N_ENC_CHUNKS = DTILES // CHUNK_DTILES
EARLY_CHUNKS = 0             # chunks streamed before/during the start barrier

_CACHE = {}


def _build(mode="full"):
    from concourse import bass, bacc, mybir, tile

    f32 = mybir.dt.float32
    Alu = mybir.AluOpType
    Act = mybir.ActivationFunctionType

    nc = bacc.Bacc(
        "TRN2",
        target_bir_lowering=False,
        debug=False,
        enable_asserts=False,
        num_devices=NCORES,
    )

    # ---- External inputs (per-core shards; same names across cores) ----
    enc = nc.dram_tensor("enc", [S_LOC, H], f32, kind="ExternalInput")
    wl = nc.dram_tensor("wl", [WROWS, H], f32, kind="ExternalInput")
    wa = nc.dram_tensor("wa", [H, WROWS], f32, kind="ExternalInput")
    bl = nc.dram_tensor("bl", [WROWS, 1], f32, kind="ExternalInput")
    ba = nc.dram_tensor("ba", [128, H // 128], f32, kind="ExternalInput")
    hidb = nc.dram_tensor("hidb", [128, H], f32, kind="ExternalInput")
    ones = nc.dram_tensor("ones", [128, 128], f32, kind="ExternalInput")
    ident = nc.dram_tensor("ident", [128, 128], f32, kind="ExternalInput")
    out_d = nc.dram_tensor("out", [RT, 128], f32, kind="ExternalOutput")

    # ---- Internal DRAM (collective bounce buffers) ----
    ep_d = nc.dram_tensor("ep_d", [H], f32)
    en_d = nc.dram_tensor("en_d", [H], f32, addr_space="Shared")
    ms_d = nc.dram_tensor("ms_d", [8], f32)
    msall_d = nc.dram_tensor("msall_d", [8 * NCORES], f32, addr_space="Shared")

    rg = [list(range(NCORES))]

    enc_r = enc.rearrange("(t p j) h -> p t (j h)", p=128, j=2)  # row = 256t + 2p + j

    with tile.TileContext(nc) as tc:
        with tc.tile_pool(name="const", bufs=1) as cpool, \
             tc.tile_pool(name="wts", bufs=1) as wpool, \
             tc.tile_pool(name="encp", bufs=1) as encpool, \
             tc.tile_pool(name="small", bufs=1) as spool, \
             tc.tile_pool(name="scratch", bufs=2) as scr, \
             tc.tile_pool(name="psb", bufs=1, space="PSUM") as ppb, \
             tc.tile_pool(name="pss", bufs=2, space="PSUM") as pps:

            # ---- SBUF tiles ----
            ones_sb = cpool.tile([128, 128], f32, tag="ones")
            ident_sb = cpool.tile([128, 128], f32, tag="ident")
            hidb_sb = wpool.tile([128, H], f32, tag="hidb")
            wl_sb = wpool.tile([WROWS, H], f32, tag="wl")
            wa_sb = wpool.tile([128, H // 128, WROWS], f32, tag="wa")
            bl_sb = wpool.tile([WROWS, 1], f32, tag="bl")
            ba_sb = wpool.tile([128, H // 128], f32, tag="ba")
            en_sb = wpool.tile([128, H], f32, tag="en")

            # Small/latency-critical loads on the SP HWDGE ring.
            nc.sync.dma_start(out=wl_sb[:], in_=wl[:])
            nc.sync.dma_start(out=bl_sb[:], in_=bl[:])
            nc.sync.dma_start(out=hidb_sb[:], in_=hidb[:])
            nc.sync.dma_start(out=wa_sb[:], in_=wa.rearrange("(t p) c -> p t c", p=128))
            nc.sync.dma_start(out=ba_sb[:], in_=ba[:])
            nc.sync.dma_start(out=ones_sb[:], in_=ones[:])
            nc.sync.dma_start(out=ident_sb[:], in_=ident[:])

            if mode == "noweights":
                # Skip h/energy stages: use hidb as a stand-in energy bcast.
                nc.vector.tensor_copy(out=en_sb[:], in_=hidb_sb[:])
            else:
                # ---- Stage 1: h chunk = W_lin[rows] @ hidden + b_lin ----
                prod1 = scr.tile([128, H], f32, tag="prod")
                hcol_raw = spool.tile([WROWS, 1], f32, tag="hcolr")
                hcol = spool.tile([WROWS, 1], f32, tag="hcol")
                nc.vector.scalar_tensor_tensor(
                    out=prod1[:], in0=wl_sb[:], scalar=1.0, in1=hidb_sb[:],
                    op0=Alu.mult, op1=Alu.mult, accum_out=hcol_raw[:],
                )
                nc.vector.tensor_add(hcol[:], hcol_raw[:], bl_sb[:])

                # h chunk -> row layout -> broadcast to 128 partitions.
                hrow_ps = pps.tile([1, 128], f32, tag="ps_small")
                nc.tensor.transpose(out=hrow_ps[:], in_=hcol[:], identity=ident_sb[:])
                hrow = spool.tile([1, 128], f32, tag="hrow")
                nc.vector.tensor_copy(out=hrow[:], in_=hrow_ps[:])
                h_bc = ppb.tile([128, 128], f32, tag="hbc")
                nc.tensor.matmul(
                    out=h_bc[:], lhsT=ones_sb[0:1, :], rhs=hrow[:],
                    start=True, stop=True,
                )

                # ---- Stage 2: partial energies ep = W_attn[:, cols] @ h[cols]
                # (column shard: all 1024 energy rows, partial over h dims).
                ep_cols = spool.tile([128, H // 128], f32, tag="epcols")
                for t in range(H // 128):
                    prod2 = scr.tile([128, WROWS], f32, tag="prodw")
                    nc.vector.scalar_tensor_tensor(
                        out=prod2[:], in0=wa_sb[:, t, :], scalar=1.0, in1=h_bc[:],
                        op0=Alu.mult, op1=Alu.mult,
                        accum_out=ep_cols[:, t:t + 1],
                    )
                # Fold in b_attn/8 so the AllReduce sum adds exactly b_attn.
                ep2 = spool.tile([128, H // 128], f32, tag="ep2")
                nc.vector.scalar_tensor_tensor(
                    out=ep2[:], in0=ba_sb[:], scalar=1.0 / NCORES, in1=ep_cols[:],
                    op0=Alu.mult, op1=Alu.add,
                )
                nc.sync.dma_start(
                    out=ep_d.rearrange("(t p) -> p t", p=128), in_=ep2[:]
                )
                cc2 = nc.gpsimd.collective_compute(
                    "AllReduce", Alu.add, replica_groups=rg,
                    ins=[ep_d[:]], outs=[en_d[:]],
                )
                en_row = spool.tile([1, H], f32, tag="enrow")
                nc.sync.dma_start(out=en_row[:], in_=en_d[:])
                en_bc = ppb.tile([128, H], f32, tag="bigbc")
                for j in range(H // 512):
                    nc.tensor.matmul(
                        out=en_bc[:, 512 * j:512 * (j + 1)],
                        lhsT=ones_sb[0:1, :],
                        rhs=en_row[0:1, 512 * j:512 * (j + 1)],
                        start=True, stop=True,
                    )
                # GEMV reads energy 32x -> keep it in SBUF.
                nc.vector.tensor_copy(out=en_sb[:], in_=en_bc[:])

                # Preload the ACT exp table off the critical path.
                dummy = spool.tile([1, 1], f32, tag="dummy")
                nc.scalar.activation(out=dummy[:], in_=en_row[0:1, 0:1], func=Act.Exp)

            # Encoder stream, split: the first EARLY_CHUNKS ride out the
            # runtime's start barrier (which in-flight DMA inflates anyway);
            # the rest are gated on the energy AllReduce so the collective
            # isn't starved by bulk DMA.
            from concourse.tile_rust import add_dep_helper
            enc_chunks = []
            for k in range(N_ENC_CHUNKS):
                t0 = k * CHUNK_DTILES
                ch = encpool.tile([128, CHUNK_DTILES, 2 * H], f32, tag=f"enc{k}")
                eng = nc.scalar if k % 2 == 0 else nc.sync
                dma = eng.dma_start(
                    out=ch[:], in_=enc_r[:, t0:t0 + CHUNK_DTILES, :]
                )
                if mode != "noweights" and k >= EARLY_CHUNKS:
                    add_dep_helper(dma.ins, cc2.ins, reason="enc after energy AR")
                enc_chunks.append(ch)

            # ---- Main GEMV: row 256t+2p+j -> ecols[p, t + DTILES*j] ----
            ecols = spool.tile([128, RT], f32, tag="ecols")
            for t in range(DTILES):
                ch = enc_chunks[t // CHUNK_DTILES]
                tt = t % CHUNK_DTILES
                for j in range(2):
                    c = t + DTILES * j
                    prod = scr.tile([128, H], f32, tag="prod")
                    nc.vector.scalar_tensor_tensor(
                        out=prod[:], in0=ch[:, tt, j * H:(j + 1) * H],
                        scalar=1.0, in1=en_sb[:],
                        op0=Alu.mult, op1=Alu.mult,
                        accum_out=ecols[:, c:c + 1],
                    )

            if mode in ("notail", "noweights"):
                # Dump raw energies (debug; permuted layout, host unperms).
                pTe = pps.tile([RT, 128], f32, tag="ps_small")
                nc.tensor.transpose(out=pTe[:], in_=ecols[:], identity=ident_sb[:])
                oute = spool.tile([RT, 128], f32, tag="oute")
                nc.vector.tensor_copy(out=oute[:], in_=pTe[:])
                nc.sync.dma_start(out=out_d[:], in_=oute[:])
            else:
                _tail(nc, mybir, spool, scr, pps, ones_sb, ident_sb,
                      ecols, ms_d, msall_d, rg, out_d, mode)

    nc.compile()
    return nc


def _tail(nc, mybir, spool, scr, pps, ones_sb, ident_sb, ecols,
          ms_d, msall_d, rg, out_d, mode="full"):
    f32 = mybir.dt.float32
    Alu = mybir.AluOpType
    Act = mybir.ActivationFunctionType

    # Local max over this core's energies.
    m1 = spool.tile([128, 1], f32, tag="m1")
    nc.vector.tensor_reduce(
        out=m1[:], in_=ecols[:], axis=mybir.AxisListType.X, op=Alu.max
    )
    mrow_ps = pps.tile([1, 128], f32, tag="ps_small")
    nc.tensor.transpose(out=mrow_ps[:], in_=m1[:], identity=ident_sb[:])
    mrow = spool.tile([1, 128], f32, tag="mrow")
    nc.vector.tensor_copy(out=mrow[:], in_=mrow_ps[:])
    mloc = spool.tile([1, 1], f32, tag="mloc")
    nc.vector.tensor_reduce(
        out=mloc[:], in_=mrow[:], axis=mybir.AxisListType.X, op=Alu.max
    )
    # -m_loc broadcast to all 128 partitions.
    mb_ps = pps.tile([128, 1], f32, tag="ps_small")
    nc.tensor.matmul(
        out=mb_ps[:], lhsT=ones_sb[0:1, :], rhs=mloc[:], start=True, stop=True,
    )
    negm = spool.tile([128, 1], f32, tag="negm")
    nc.vector.tensor_scalar_mul(negm[:], mb_ps[:], -1.0)

    # p = exp(e - m_loc), rowsum = per-partition sums.
    pcols = spool.tile([128, RT], f32, tag="pcols")
    rowsum = spool.tile([128, 1], f32, tag="rowsum")
    nc.scalar.activation(
        out=pcols[:], in_=ecols[:], func=Act.Exp,
        bias=negm[:], scale=1.0, accum_out=rowsum[:],
    )
    # s_loc = sum over partitions (ones-matmul).
    s_ps = pps.tile([1, 1], f32, tag="ps_small")
    nc.tensor.matmul(
        out=s_ps[:], lhsT=ones_sb[:, 0:1], rhs=rowsum[:], start=True, stop=True,
    )
    if mode == "tail1":
        # Stop before the 3rd collective: dump exp(e - m_loc), stash s_loc.
        pT1 = pps.tile([RT, 128], f32, tag="ps_small")
        nc.tensor.transpose(out=pT1[:], in_=pcols[:], identity=ident_sb[:])
        oute = spool.tile([RT, 128], f32, tag="oute")
        nc.vector.tensor_copy(out=oute[:], in_=pT1[:])
        nc.vector.tensor_copy(out=oute[0:1, 127:128], in_=s_ps[:])
        nc.sync.dma_start(out=out_d[:], in_=oute[:])
        return
    # Pack (m_loc, s_loc) (padded to 32B) and AllGather the pairs.
    ms = spool.tile([1, 8], f32, tag="ms")
    nc.vector.memset(ms[:], 0.0)
    nc.vector.tensor_copy(out=ms[0:1, 0:1], in_=mloc[:])
    nc.vector.tensor_copy(out=ms[0:1, 1:2], in_=s_ps[:])
    nc.sync.dma_start(out=ms_d[:], in_=ms[:])
    nc.gpsimd.collective_compute(
        "AllGather", Alu.bypass, replica_groups=rg,
        ins=[ms_d[:]], outs=[msall_d[:]],
    )
    msall = spool.tile([1, NCORES, 8], f32, tag="msall")
    nc.sync.dma_start(out=msall[:], in_=msall_d[:])

    # M = max_c m_c ; Z = sum_c s_c * exp(m_c - M)
    mg = spool.tile([1, 1], f32, tag="mg")
    nc.vector.tensor_reduce(
        out=mg[:], in_=msall[0:1, :, 0:1], axis=mybir.AxisListType.XY, op=Alu.max,
    )
    negmg = spool.tile([1, 1], f32, tag="negmg")
    nc.vector.tensor_scalar_mul(negmg[:], mg[:], -1.0)
    if mode == "tail2":
        pT2 = pps.tile([RT, 128], f32, tag="ps_small")
        nc.tensor.transpose(out=pT2[:], in_=pcols[:], identity=ident_sb[:])
        oute = spool.tile([RT, 128], f32, tag="oute")
        nc.vector.tensor_copy(out=oute[:], in_=pT2[:])
        nc.vector.tensor_copy(out=oute[0:1, 126:127], in_=negmg[:])
        nc.sync.dma_start(out=out_d[:], in_=oute[:])
        return
    wexp = spool.tile([1, NCORES], f32, tag="wexp")
    nc.scalar.activation(
        out=wexp[:], in_=msall[0:1, :, 0:1], func=Act.Exp,
        bias=negmg[:], scale=1.0,
    )
    zscr = scr.tile([1, NCORES], f32, tag="zscr")
    zz = spool.tile([1, 1], f32, tag="zz")
    nc.vector.scalar_tensor_tensor(
        out=zscr[:], in0=wexp[:], scalar=1.0, in1=msall[0:1, :, 1:2],
        op0=Alu.mult, op1=Alu.mult, accum_out=zz[:],
    )
    invz = spool.tile([1, 1], f32, tag="invz")
    nc.vector.reciprocal(invz[:], zz[:])
    # This core's rescale factor: exp(m_loc - M) / Z.
    wme = spool.tile([1, 1], f32, tag="wme")
    nc.scalar.activation(
        out=wme[:], in_=mloc[:], func=Act.Exp, bias=negmg[:], scale=1.0,
    )
    scale_me = spool.tile([1, 1], f32, tag="scme")
    nc.vector.tensor_mul(scale_me[:], wme[:], invz[:])
    if mode == "tail3":
        pT3 = pps.tile([RT, 128], f32, tag="ps_small")
        nc.tensor.transpose(out=pT3[:], in_=pcols[:], identity=ident_sb[:])
        oute = spool.tile([RT, 128], f32, tag="oute")
        nc.vector.tensor_copy(out=oute[:], in_=pT3[:])
        nc.vector.tensor_copy(out=oute[0:1, 126:127], in_=scale_me[:])
        nc.sync.dma_start(out=out_d[:], in_=oute[:])
        return
    # Broadcast scale to RT partitions (for the transposed layout).
    scb_ps = pps.tile([RT, 1], f32, tag="ps_small")
    nc.tensor.matmul(
        out=scb_ps[:], lhsT=ones_sb[0:1, 0:RT], rhs=scale_me[:],
        start=True, stop=True,
    )
    scb = spool.tile([RT, 1], f32, tag="scb")
    nc.vector.tensor_copy(out=scb[:], in_=scb_ps[:])
    if mode == "tail4":
        pT4 = pps.tile([RT, 128], f32, tag="ps_small")
        nc.tensor.transpose(out=pT4[:], in_=pcols[:], identity=ident_sb[:])
        oute = spool.tile([RT, 128], f32, tag="oute")
        nc.vector.tensor_copy(out=oute[:], in_=pT4[:])
        nc.vector.tensor_copy(out=oute[0:RT, 126:127], in_=scb[:])
        nc.sync.dma_start(out=out_d[:], in_=oute[:])
        return

    # Transpose pcols, scale, store contiguously in the permuted (c, p)
    # layout; the host unshard applies the inverse row permutation
    # (row = 256t + 2p + j lives at out[t + DTILES*j, p]).
    pT_ps = pps.tile([RT, 128], f32, tag="ps_small")
    nc.tensor.transpose(out=pT_ps[:], in_=pcols[:], identity=ident_sb[:])
    out_sb = spool.tile([RT, 128], f32, tag="outsb")
    nc.scalar.mul(out_sb[:], pT_ps[:], scb[:])
    nc.sync.dma_start(out=out_d[:], in_=out_sb[:])


def _shard_perm():
    # perm[local_row] = flat index into the core's [RT, 128] output buffer.
    if "perm" not in _CACHE:
        t = np.arange(S_LOC) // 256
        p = (np.arange(S_LOC) % 256) // 2
        j = np.arange(S_LOC) % 2
        _CACHE["perm"] = ((t + DTILES * j) * 128 + p).astype(np.int64)
    return _CACHE["perm"]


def _get_nc():
    if "nc" not in _CACHE:
        _CACHE["nc"] = _build()
    return _CACHE["nc"]


def _make_in_maps(hidden, encoder_states, W_lin, b_lin, W_attn, b_attn):
    hidden = np.asarray(hidden, dtype=np.float32)
    encoder_states = np.ascontiguousarray(np.asarray(encoder_states, dtype=np.float32))
    W_lin = np.ascontiguousarray(np.asarray(W_lin, dtype=np.float32))
    W_attn = np.ascontiguousarray(np.asarray(W_attn, dtype=np.float32))
    b_lin = np.asarray(b_lin, dtype=np.float32)
    b_attn = np.asarray(b_attn, dtype=np.float32)

    hidb = np.ascontiguousarray(np.broadcast_to(hidden[None, :], (128, H)))
    ones = np.ones((128, 128), dtype=np.float32)
    ident = np.eye(128, dtype=np.float32)

    in_maps = []
    for c in range(NCORES):
        in_maps.append({
            "enc": encoder_states[c * S_LOC:(c + 1) * S_LOC],
            "wl": W_lin[c * WROWS:(c + 1) * WROWS],
            "wa": np.ascontiguousarray(W_attn[:, c * WROWS:(c + 1) * WROWS]),
            "bl": np.ascontiguousarray(b_lin[c * WROWS:(c + 1) * WROWS])[:, None],
            "ba": np.ascontiguousarray(b_attn.reshape(H // 128, 128).T),
            "hidb": hidb,
            "ones": ones,
            "ident": ident,
        })
    return in_maps


def kernel(hidden, encoder_states, W_lin, b_lin, W_attn, b_attn):
    from concourse.bass_utils import run_bass_kernel_spmd

    nc = _get_nc()
    in_maps = _make_in_maps(hidden, encoder_states, W_lin, b_lin, W_attn, b_attn)
    res = run_bass_kernel_spmd(nc, in_maps, core_ids=list(range(NCORES)))
    perm = _shard_perm()
    parts = [res.results[c]["out"].reshape(-1)[perm] for c in range(NCORES)]
    return np.concatenate(parts).astype(np.float32)[:, None]
